# revision 12
# baseline (speedup 1.0000x reference)
"""Bass/Trainium2 kernel for nn_CenterBasedLoss (fused segment-mean + EMA update).

Strategy (data-parallel over N, 8 NeuronCores):
  - Each core gets a 32768-row shard of features/labels.
  - One-hot matmul on the TensorEngine: for each 128-row tile, build a
    [128, 1000] fp16 one-hot from the labels (iota + is_equal on the DVE),
    then 8 matmuls (one per 125-class chunk) accumulate
    one_hot.T @ [features | 1] into PSUM -> per-core partial sums+counts.
  - ReduceScatter(add) across the 8 cores: core i receives the reduced
    [125, 257] partial for class chunk i.
  - Each core computes the EMA update for its own 125 classes and writes a
    [125, 256] output slice; the host concatenates the 8 slices.
"""

import sys

if "/opt/trn_rl_repo" not in sys.path:
    sys.path.insert(0, "/opt/trn_rl_repo")

import numpy as np

from concourse import bacc, mybir
from concourse import bass_utils
import concourse.tile as tile

N_CORES = 8
N = 262144
D = 256
C = 1000
ALPHA = 0.5

SHARD = N // N_CORES            # 32768 rows per core
P = 128                         # SBUF partitions / matmul contraction
TILES = SHARD // P              # 256 row-tiles per core
SUP = 8                         # row-tiles per DMA super-tile (1 MiB loads)
NSUP = TILES // SUP             # 32 super-tiles
CCHUNK = C // N_CORES           # 125 classes per chunk/core
DP1 = D + 1                     # features + ones column

_nc_cache = None

IMPL = "v2"  # "v1" (fp16, option A) or "v2" (fp8 DoubleRow, option B)


def _build(with_collective=True, repeat=1):
    nc = bacc.Bacc("TRN2", target_bir_lowering=False, debug=False,
                   enable_asserts=True,
                   num_devices=N_CORES if with_collective else 1)
    f32 = mybir.dt.float32
    f16 = mybir.dt.float16
    i16 = mybir.dt.int16

    feat_d = nc.dram_tensor("features", [SHARD, D], f32, kind="ExternalInput").ap()
    # labels transposed on host: labels_t[p, t] = labels[t*128 + p], as f32
    lab_d = nc.dram_tensor("labels_t", [P, TILES], f32, kind="ExternalInput").ap()
    cen_d = nc.dram_tensor("centers", [CCHUNK, D], f32, kind="ExternalInput").ap()
    out_d = nc.dram_tensor("out", [CCHUNK, D], f32, kind="ExternalOutput").ap()

    with tile.TileContext(nc) as tc:
        with tc.tile_pool(name="const", bufs=1) as const, \
             tc.tile_pool(name="f32p", bufs=3) as f32p, \
             tc.tile_pool(name="f16p", bufs=3) as f16p, \
             tc.tile_pool(name="ohp", bufs=4) as ohp, \
             tc.tile_pool(name="tailp", bufs=1) as tailp, \
             tc.tile_pool(name="psum", bufs=1, space="PSUM") as psum, \
             tc.tile_pool(name="dram", bufs=1, space="DRAM") as dram:

            # --- constants ---
            iota_i = const.tile([P, C], i16, tag="iota_i")
            nc.gpsimd.iota(iota_i[:], pattern=[[1, C]], base=0, channel_multiplier=0)
            iota_f = const.tile([P, C], f16, tag="iota_f")
            nc.vector.tensor_copy(out=iota_f[:], in_=iota_i[:])

            labels_sb = const.tile([P, TILES], f32, tag="labels")
            nc.sync.dma_start(out=labels_sb[:], in_=lab_d[:])

            # --- per-class-chunk PSUM accumulators [125, 257] ---
            accs = [psum.tile([CCHUNK, DP1], f32, tag=f"acc{c}", name=f"acc{c}")
                    for c in range(N_CORES)]

            # --- main accumulation loop ---
            for r in range(repeat):
                for s in range(NSUP):
                    ft32 = f32p.tile([P, SUP, D], f32, tag="ft32", name="ft32")
                    src = feat_d[s * SUP * P:(s + 1) * SUP * P, :]
                    nc.sync.dma_start(out=ft32[:],
                                      in_=src.rearrange("(j p) d -> p j d", p=P))

                    ft16 = f16p.tile([P, SUP, DP1 + 3], f16, tag="ft16", name="ft16")
                    nc.scalar.activation(out=ft16[:, :, 0:D], in_=ft32[:],
                                         func=mybir.ActivationFunctionType.Copy)
                    nc.gpsimd.memset(ft16[:, :, D:DP1], 1.0)

                    for j in range(SUP):
                        t = s * SUP + j
                        oh = ohp.tile([P, C], f16, tag="oh", name="oh")
                        nc.vector.tensor_scalar(
                            out=oh[:], in0=iota_f[:],
                            scalar1=labels_sb[:, t:t + 1], scalar2=None,
                            op0=mybir.AluOpType.is_equal,
                        )
                        for c in range(N_CORES):
                            nc.tensor.matmul(
                                out=accs[c][:, :],
                                lhsT=oh[:, c * CCHUNK:(c + 1) * CCHUNK],
                                rhs=ft16[:, j, 0:DP1],
                                start=(r == 0 and t == 0),
                                stop=(r == repeat - 1 and t == TILES - 1),
                            )

            # --- partials -> DRAM bounce, ReduceScatter across cores ---
            bounce_in = dram.tile([C, DP1], f32)
            bounce_out = dram.tile([CCHUNK, DP1], f32)
            for c in range(N_CORES):
                ps = tailp.tile([CCHUNK, DP1], f32, tag=f"ps{c}")
                nc.vector.tensor_copy(out=ps[:], in_=accs[c][:])
                nc.sync.dma_start(out=bounce_in[c * CCHUNK:(c + 1) * CCHUNK, :], in_=ps[:])
            if with_collective:
                nc.gpsimd.collective_compute(
                    "ReduceScatter",
                    mybir.AluOpType.add,
                    replica_groups=[list(range(N_CORES))],
                    ins=[bounce_in.opt()],
                    outs=[bounce_out.opt()],
                )
            else:  # single-core modeling variant
                nc.sync.dma_start(out=bounce_out[:], in_=bounce_in[0:CCHUNK, :])

            # --- EMA tail for this core's 125 classes ---
            red = tailp.tile([CCHUNK, DP1], f32, tag="red")
            nc.sync.dma_start(out=red[:], in_=bounce_out[:])
            cen = tailp.tile([CCHUNK, D], f32, tag="cen")
            nc.sync.dma_start(out=cen[:], in_=cen_d[:])

            counts = red[:, D:DP1]
            sums = red[:, 0:D]
            s_t = tailp.tile([CCHUNK, 1], f32, tag="s_t")
            # s = (counts > 0) * ALPHA
            nc.vector.tensor_scalar(out=s_t[:], in0=counts, scalar1=0.0, scalar2=ALPHA,
                                    op0=mybir.AluOpType.is_gt, op1=mybir.AluOpType.mult)
            safe = tailp.tile([CCHUNK, 1], f32, tag="safe")
            nc.vector.tensor_scalar_max(out=safe[:], in0=counts, scalar1=1.0)
            recip = tailp.tile([CCHUNK, 1], f32, tag="recip")
            nc.vector.reciprocal(out=recip[:], in_=safe[:])
            rs_t = tailp.tile([CCHUNK, 1], f32, tag="rs_t")
            nc.vector.tensor_mul(out=rs_t[:], in0=recip[:], in1=s_t[:])
            om_s = tailp.tile([CCHUNK, 1], f32, tag="om_s")
            # 1 - s
            nc.vector.tensor_scalar(out=om_s[:], in0=s_t[:], scalar1=-1.0, scalar2=1.0,
                                    op0=mybir.AluOpType.mult, op1=mybir.AluOpType.add)
            m_sb = tailp.tile([CCHUNK, D], f32, tag="m_sb")
            nc.vector.tensor_scalar_mul(out=m_sb[:], in0=sums, scalar1=rs_t[:])
            out_sb = tailp.tile([CCHUNK, D], f32, tag="out_sb")
            # out = centers * (1 - s) + (s/safe) * sums
            nc.vector.scalar_tensor_tensor(out=out_sb[:], in0=cen[:], scalar=om_s[:],
                                           in1=m_sb[:], op0=mybir.AluOpType.mult,
                                           op1=mybir.AluOpType.add)
            nc.sync.dma_start(out=out_d[:], in_=out_sb[:])

    nc.compile()
    return nc


def _build_v2(with_collective=True, repeat=1):
    """fp8e4 + DoubleRow variant: features are the stationary operand
    ([128, 2, 128] k-pair chunks, K=256 rows per matmul), the one-hot is the
    moving operand ([128, 2, 500] per class half). PSUM accumulates
    [feat, class] partials plus a [1, class] count row; a PE-transpose tail
    rearranges to class-major before the ReduceScatter."""
    from concourse.masks import make_identity

    nc = bacc.Bacc("TRN2", target_bir_lowering=False, debug=False,
                   enable_asserts=True,
                   num_devices=N_CORES if with_collective else 1)
    f32 = mybir.dt.float32
    f16 = mybir.dt.float16
    f8 = mybir.dt.float8e4
    i16 = mybir.dt.int16

    NDR = TILES // 2          # 128 double-row tiles of 256 rows
    DRS = SUP // 2            # 4 double-row tiles per super-tile
    HC = C // 2               # 500 classes per PSUM half
    OHW = 1008                # padded one-hot row width (16B-aligned k-tile stride)

    feat_d = nc.dram_tensor("features", [SHARD, D], f32, kind="ExternalInput").ap()
    lab_d = nc.dram_tensor("labels_t", [P, TILES], f32, kind="ExternalInput").ap()
    cen_d = nc.dram_tensor("centers", [CCHUNK, D], f32, kind="ExternalInput").ap()
    out_d = nc.dram_tensor("out", [CCHUNK, D], f32, kind="ExternalOutput").ap()

    with tile.TileContext(nc) as tc:
        with tc.tile_pool(name="const", bufs=1) as const, \
             tc.tile_pool(name="f32p", bufs=4) as f32p, \
             tc.tile_pool(name="f8p", bufs=3) as f8p, \
             tc.tile_pool(name="ohp", bufs=6) as ohp, \
             tc.tile_pool(name="tailp", bufs=1) as tailp, \
             tc.tile_pool(name="stagep", bufs=2) as stagep, \
             tc.tile_pool(name="psum", bufs=1, space="PSUM") as psum, \
             tc.tile_pool(name="tpp", bufs=2, space="PSUM") as tpp, \
             tc.tile_pool(name="dram", bufs=1, space="DRAM") as dram:

            # --- constants ---
            iota_i = const.tile([P, C], i16, tag="iota_i")
            nc.gpsimd.iota(iota_i[:], pattern=[[1, C]], base=0, channel_multiplier=0)
            iota_f = const.tile([P, C], f16, tag="iota_f")
            nc.vector.tensor_copy(out=iota_f[:], in_=iota_i[:])
            labels_sb = const.tile([P, TILES], f32, tag="labels")
            nc.sync.dma_start(out=labels_sb[:], in_=lab_d[:])
            ones8 = const.tile([P, 2, 16], f8, tag="ones8")
            nc.gpsimd.memset(ones8[:], 1.0)
            ident = const.tile([P, P], f32, tag="ident")
            make_identity(nc, ident[:])

            # --- PSUM accumulators: [feat_chunk, class_half] + count rows ---
            pa = [[psum.tile([P, HC], f32, tag=f"pa{m}{h}", name=f"pa{m}{h}")
                   for h in range(2)] for m in range(2)]
            pc = [psum.tile([1, HC], f32, tag=f"pc{h}", name=f"pc{h}") for h in range(2)]

            # --- main accumulation loop ---
            for r in range(repeat):
                for s in range(NSUP):
                    ft32 = f32p.tile([P, SUP, D], f32, tag="ft32", name="ft32")
                    src = feat_d[s * SUP * P:(s + 1) * SUP * P, :]
                    ft8 = f8p.tile([P, SUP, D], f8, tag="ft8", name="ft8")
                    if r == 0 and s == 0:
                        # split the first load so the pipeline starts sooner
                        for q in range(4):
                            qs = slice(q * SUP // 4, (q + 1) * SUP // 4)
                            nc.sync.dma_start(
                                out=ft32[:, qs, :],
                                in_=src.rearrange("(j p) d -> p j d", p=P)[:, qs, :])
                            nc.scalar.activation(
                                out=ft8[:, qs, :], in_=ft32[:, qs, :],
                                func=mybir.ActivationFunctionType.Copy)
                    else:
                        nc.sync.dma_start(out=ft32[:],
                                          in_=src.rearrange("(j p) d -> p j d", p=P))
                        nc.scalar.activation(out=ft8[:], in_=ft32[:],
                                             func=mybir.ActivationFunctionType.Copy)

                    for k in range(DRS):
                        dr = s * DRS + k
                        oh8 = ohp.tile([P, 2, OHW], f8, tag="oh8", name="oh8")
                        for j in range(2):
                            t = s * SUP + 2 * k + j
                            nc.vector.tensor_scalar(
                                out=oh8[:, j, 0:C], in0=iota_f[:],
                                scalar1=labels_sb[:, t:t + 1], scalar2=None,
                                op0=mybir.AluOpType.is_equal,
                            )
                        first = (r == 0 and dr == 0)
                        last = (r == repeat - 1 and dr == NDR - 1)
                        for h in range(2):
                            rhs = oh8[:, :, h * HC:(h + 1) * HC]
                            for m in range(2):
                                nc.tensor.matmul(
                                    out=pa[m][h][:],
                                    lhsT=ft8[:, 2 * k:2 * k + 2, m * P:(m + 1) * P],
                                    rhs=rhs,
                                    perf_mode=mybir.MatmulPerfMode.DoubleRow,
                                    start=first, stop=last,
                                )
                            nc.tensor.matmul(
                                out=pc[h][:],
                                lhsT=ones8[:, :, 0:1],
                                rhs=rhs,
                                perf_mode=mybir.MatmulPerfMode.DoubleRow,
                                start=first, stop=last,
                            )

            # --- PSUM -> SBUF ([feat, class] + counts) ---
            sum_a = tailp.tile([P, C], f32, tag="sum_a")   # feats 0:128
            sum_b = tailp.tile([P, C], f32, tag="sum_b")   # feats 128:256
            cnt_sb = tailp.tile([1, C], f32, tag="cnt_sb")
            for h in range(2):
                sl = slice(h * HC, (h + 1) * HC)
                nc.vector.tensor_copy(out=sum_a[:, sl], in_=pa[0][h][:])
                nc.vector.tensor_copy(out=sum_b[:, sl], in_=pa[1][h][:])
                nc.vector.tensor_copy(out=cnt_sb[:, sl], in_=pc[h][:])

            # --- ReduceScatter in feat-major [8, 257, 125] (rank i gets its
            # 125-class chunk); transpose to class-major AFTER the collective ---
            bounce_in = dram.tile([N_CORES, DP1, CCHUNK], f32)
            bounce_out = dram.tile([DP1, CCHUNK], f32)
            for cj in range(N_CORES):
                sl = slice(cj * CCHUNK, (cj + 1) * CCHUNK)
                nc.sync.dma_start(out=bounce_in[cj, 0:P, :], in_=sum_a[:, sl])
                nc.sync.dma_start(out=bounce_in[cj, P:D, :], in_=sum_b[:, sl])
                nc.sync.dma_start(out=bounce_in[cj, D:DP1, :], in_=cnt_sb[:, sl])

            if with_collective:
                nc.gpsimd.collective_compute(
                    "ReduceScatter",
                    mybir.AluOpType.add,
                    replica_groups=[list(range(N_CORES))],
                    ins=[bounce_in.opt()],
                    outs=[bounce_out.opt()],
                )
            else:  # single-core modeling variant
                nc.sync.dma_start(out=bounce_out[:], in_=bounce_in[0, :, :])

            # --- transpose the reduced [257, 125] chunk to class-major ---
            red_a = stagep.tile([P, CCHUNK], f32, tag="red_a", name="red_a")
            nc.sync.dma_start(out=red_a[:], in_=bounce_out[0:P, :])
            red_b = stagep.tile([P, CCHUNK], f32, tag="red_b", name="red_b")
            nc.sync.dma_start(out=red_b[:], in_=bounce_out[P:D, :])
            red_c = stagep.tile([1, CCHUNK], f32, tag="red_c", name="red_c")
            nc.sync.dma_start(out=red_c[:], in_=bounce_out[D:DP1, :])

            red = tailp.tile([CCHUNK, DP1], f32, tag="red")
            tpa = tpp.tile([CCHUNK, P], f32, tag="tp", name="tpa")
            nc.tensor.transpose(out=tpa[:], in_=red_a[:], identity=ident[:])
            nc.vector.tensor_copy(out=red[:, 0:P], in_=tpa[:])
            tpb = tpp.tile([CCHUNK, P], f32, tag="tp", name="tpb")
            nc.tensor.transpose(out=tpb[:], in_=red_b[:], identity=ident[:])
            nc.vector.tensor_copy(out=red[:, P:D], in_=tpb[:])
            tpc = tpp.tile([CCHUNK, P], f32, tag="tp", name="tpc")
            nc.tensor.transpose(out=tpc[:], in_=red_c[:], identity=ident[0:1, :])
            nc.vector.tensor_copy(out=red[:, D:DP1], in_=tpc[:, 0:1])

            cen = tailp.tile([CCHUNK, D], f32, tag="cen")
            nc.sync.dma_start(out=cen[:], in_=cen_d[:])

            counts = red[:, D:DP1]
            sums = red[:, 0:D]
            s_t = tailp.tile([CCHUNK, 1], f32, tag="s_t")
            nc.vector.tensor_scalar(out=s_t[:], in0=counts, scalar1=0.0, scalar2=ALPHA,
                                    op0=mybir.AluOpType.is_gt, op1=mybir.AluOpType.mult)
            safe = tailp.tile([CCHUNK, 1], f32, tag="safe")
            nc.vector.tensor_scalar_max(out=safe[:], in0=counts, scalar1=1.0)
            recip = tailp.tile([CCHUNK, 1], f32, tag="recip")
            nc.vector.reciprocal(out=recip[:], in_=safe[:])
            rs_t = tailp.tile([CCHUNK, 1], f32, tag="rs_t")
            nc.vector.tensor_mul(out=rs_t[:], in0=recip[:], in1=s_t[:])
            om_s = tailp.tile([CCHUNK, 1], f32, tag="om_s")
            nc.vector.tensor_scalar(out=om_s[:], in0=s_t[:], scalar1=-1.0, scalar2=1.0,
                                    op0=mybir.AluOpType.mult, op1=mybir.AluOpType.add)
            m_sb = tailp.tile([CCHUNK, D], f32, tag="m_sb")
            nc.vector.tensor_scalar_mul(out=m_sb[:], in0=sums, scalar1=rs_t[:])
            out_sb = tailp.tile([CCHUNK, D], f32, tag="out_sb")
            nc.vector.scalar_tensor_tensor(out=out_sb[:], in0=cen[:], scalar=om_s[:],
                                           in1=m_sb[:], op0=mybir.AluOpType.mult,
                                           op1=mybir.AluOpType.add)
            nc.sync.dma_start(out=out_d[:], in_=out_sb[:])

    nc.compile()
    return nc


def _get_nc():
    global _nc_cache
    if _nc_cache is None:
        _nc_cache = _build_v2() if IMPL == "v2" else _build()
    return _nc_cache


def kernel(features, labels, centers, **_ignored):
    features = np.ascontiguousarray(np.asarray(features, dtype=np.float32))
    labels = np.asarray(labels)
    centers = np.ascontiguousarray(np.asarray(centers, dtype=np.float32))
    assert features.shape == (N, D) and centers.shape == (C, D)

    labels_f = labels.astype(np.float32)
    nc = _get_nc()
    in_maps = []
    for i in range(N_CORES):
        fsh = features[i * SHARD:(i + 1) * SHARD]
        lsh = labels_f[i * SHARD:(i + 1) * SHARD]
        lab_t = np.ascontiguousarray(lsh.reshape(TILES, P).T)  # [128, 256]
        csh = centers[i * CCHUNK:(i + 1) * CCHUNK]
        in_maps.append({"features": fsh, "labels_t": lab_t, "centers": csh})

    res = bass_utils.run_bass_kernel_spmd(nc, in_maps, core_ids=list(range(N_CORES)))
    out = np.concatenate([np.asarray(res.results[i]["out"]) for i in range(N_CORES)],
                         axis=0)
    return out.astype(np.float32)


def profile_exec_ns(tmpdir=None):
    """Run once more with NTFF tracing; return exec_time_ns (or None)."""
    rng = np.random.default_rng(0)
    features = rng.standard_normal((N, D)).astype(np.float32)
    labels = rng.integers(0, C, size=(N,))
    centers = rng.standard_normal((C, D)).astype(np.float32)
    labels_f = labels.astype(np.float32)
    nc = _get_nc()
    in_maps = []
    for i in range(N_CORES):
        fsh = features[i * SHARD:(i + 1) * SHARD]
        lsh = labels_f[i * SHARD:(i + 1) * SHARD]
        lab_t = np.ascontiguousarray(lsh.reshape(TILES, P).T)
        csh = centers[i * CCHUNK:(i + 1) * CCHUNK]
        in_maps.append({"features": fsh, "labels_t": lab_t, "centers": csh})
    res = bass_utils.run_bass_kernel_spmd(nc, in_maps, core_ids=list(range(N_CORES)),
                                          trace=True, tmpdir=tmpdir)
    return res.exec_time_ns
